# revision 6
# baseline (speedup 1.0000x reference)
"""Mixtral sparse-MoE block on 8 Trainium2 NeuronCores.

Strategy (expert-parallel, dense):
  - Core c holds expert c's weights (w1/w3/w2), x is replicated.
  - Every core computes router logits + softmax top-2 combine weights
    on device (fp32 gate matmul; exp/max/mask DVE ops).
  - Every core runs the SwiGLU FFN for its expert over all T=1024
    tokens in bf16 (fp32 accumulate in PSUM), scales rows by its
    combine weight column, and a ReduceScatter(add) sums the 8 partial
    outputs, leaving each core with a 128-token shard of the final
    output.  Host concatenates the shards (pure unshard).
  - Router logits output is taken from core 0.

Layouts are host-prepared so every matmul operand lands with its
contraction dim on SBUF partitions; no on-device transposes needed.
"""

import numpy as np
import ml_dtypes

E = 8
TOP_K = 2
H = 1024
F = 3584
B, S = 2, 512
T = B * S            # 1024 tokens
P = 128
NCORES = 8
HJ = H // P          # 8  H-chunks
FG = F // P          # 28 F-chunks
TT = T // P          # 8  token tiles
TSHARD = T // NCORES # 128 tokens per output shard

_BF16 = ml_dtypes.bfloat16

_compiled = {}


def _build_nc(collective=True):
    import concourse.bass as bass
    import concourse.mybir as mybir
    import concourse.tile as tile
    from concourse import bacc
    from contextlib import ExitStack

    f32 = mybir.dt.float32
    bf16 = mybir.dt.bfloat16
    nc = bacc.Bacc(
        "TRN2",
        target_bir_lowering=False,
        debug=False,
        num_devices=NCORES,
    )

    # I/O (per-core data supplied via in_maps)
    xT_bf_d = nc.dram_tensor("xT_bf", [P, HJ, T], bf16, kind="ExternalInput")
    xT_f32_d = nc.dram_tensor("xT_f32", [P, HJ, T], f32, kind="ExternalInput")
    gate_d = nc.dram_tensor("gateT", [P, HJ, E], f32, kind="ExternalInput")
    onehot_d = nc.dram_tensor("onehot", [P, E], f32, kind="ExternalInput")
    w1_d = nc.dram_tensor("w1c", [P, FG, HJ, P], bf16, kind="ExternalInput")
    w3_d = nc.dram_tensor("w3c", [P, FG, HJ, P], bf16, kind="ExternalInput")
    w2_d = nc.dram_tensor("w2c", [P, FG, H], bf16, kind="ExternalInput")

    logits_out = nc.dram_tensor("logits_out", [P, TT, E], f32, kind="ExternalOutput")
    final_shard = nc.dram_tensor("final_shard", [TSHARD, H], f32, kind="ExternalOutput")

    with tile.TileContext(nc) as tc, ExitStack() as ctx:
        const = ctx.enter_context(tc.tile_pool(name="const", bufs=1))
        wpool = ctx.enter_context(tc.tile_pool(name="wpool", bufs=3))
        xfpool = ctx.enter_context(tc.tile_pool(name="xfpool", bufs=3))
        tmps = ctx.enter_context(tc.tile_pool(name="tmps", bufs=3))
        small = ctx.enter_context(tc.tile_pool(name="small", bufs=3))
        psum = ctx.enter_context(tc.tile_pool(name="psum", bufs=2, space="PSUM"))
        psum_s = ctx.enter_context(tc.tile_pool(name="psum_s", bufs=2, space="PSUM"))
        dram = ctx.enter_context(tc.tile_pool(name="dram", bufs=1, space="DRAM"))

        # Resident SBUF tensors
        xT_sb = const.tile([P, HJ, T], bf16)
        nc.sync.dma_start(xT_sb[:], xT_bf_d[:])
        gate_sb = const.tile([P, HJ, E], f32)
        nc.sync.dma_start(gate_sb[:], gate_d[:])
        onehot_sb = const.tile([P, E], f32)
        nc.sync.dma_start(onehot_sb[:], onehot_d[:])
        w2_sb = const.tile([P, FG, H], bf16)
        nc.sync.dma_start(w2_sb[:], w2_d[:])
        aT = const.tile([P, FG, T], bf16)           # silu(x@w1.T)*(x@w3.T), F on partitions
        comb = const.tile([P, TT], f32)             # this core's combine weight per token

        # ---- Phase 1: router logits + top-2 combine weights ----
        for j in range(TT):
            ps_l = psum_s.tile([P, E], f32)
            for k in range(HJ):
                xf = xfpool.tile([P, P], f32)
                nc.sync.dma_start(xf[:], xT_f32_d[:, k, j * P:(j + 1) * P])
                nc.tensor.matmul(ps_l[:], xf[:], gate_sb[:, k, :],
                                 start=(k == 0), stop=(k == HJ - 1))
            lg = small.tile([P, E], f32)
            nc.vector.tensor_copy(lg[:], ps_l[:])
            nc.sync.dma_start(logits_out[:, j, :], lg[:])

            m = small.tile([P, 1], f32)
            nc.vector.reduce_max(m[:], lg[:], axis=mybir.AxisListType.X)
            negm = small.tile([P, 1], f32)
            nc.vector.tensor_scalar_mul(negm[:], m[:], -1.0)
            pexp = small.tile([P, E], f32)
            nc.scalar.activation(pexp[:], lg[:], mybir.ActivationFunctionType.Exp,
                                 bias=negm[:])
            v1 = small.tile([P, 1], f32)
            nc.vector.reduce_max(v1[:], pexp[:], axis=mybir.AxisListType.X)
            eq1 = small.tile([P, E], f32)
            nc.vector.tensor_tensor(eq1[:], pexp[:], v1.to_broadcast([P, E]),
                                    mybir.AluOpType.is_equal)
            pm = small.tile([P, E], f32)
            nc.vector.tensor_tensor(pm[:], pexp[:], eq1[:], mybir.AluOpType.mult)
            pm2 = small.tile([P, E], f32)
            nc.vector.tensor_tensor(pm2[:], pexp[:], pm[:], mybir.AluOpType.subtract)
            v2 = small.tile([P, 1], f32)
            nc.vector.reduce_max(v2[:], pm2[:], axis=mybir.AxisListType.X)
            eq2 = small.tile([P, E], f32)
            nc.vector.tensor_tensor(eq2[:], pm2[:], v2.to_broadcast([P, E]),
                                    mybir.AluOpType.is_equal)
            sel = small.tile([P, E], f32)
            nc.vector.tensor_tensor(sel[:], eq1[:], eq2[:], mybir.AluOpType.add)
            wsel = small.tile([P, E], f32)
            nc.vector.tensor_tensor(wsel[:], pexp[:], sel[:], mybir.AluOpType.mult)
            denom = small.tile([P, 1], f32)
            nc.vector.tensor_tensor(denom[:], v1[:], v2[:], mybir.AluOpType.add)
            inv = small.tile([P, 1], f32)
            nc.vector.reciprocal(inv[:], denom[:])
            wnorm = small.tile([P, E], f32)
            nc.vector.tensor_scalar_mul(wnorm[:], wsel[:], inv[:])
            wmine = small.tile([P, E], f32)
            nc.vector.tensor_tensor(wmine[:], wnorm[:], onehot_sb[:],
                                    mybir.AluOpType.mult)
            nc.vector.reduce_sum(comb[:, j:j + 1], wmine[:], axis=mybir.AxisListType.X)

        # ---- Phase 2: h1 = x@w1.T, h3 = x@w3.T, aT = silu(h1)*h3  (F on partitions) ----
        for f in range(FG):
            w1t = wpool.tile([P, HJ, P], bf16, tag="w1t")
            nc.sync.dma_start(w1t[:], w1_d[:, f, :, :])
            w3t = wpool.tile([P, HJ, P], bf16, tag="w3t")
            nc.sync.dma_start(w3t[:], w3_d[:, f, :, :])
            for th in range(2):
                ts_ = slice(th * 512, (th + 1) * 512)
                ps1 = psum.tile([P, 512], f32, tag="ps1")
                for k in range(HJ):
                    nc.tensor.matmul(ps1[:], w1t[:, k, :], xT_sb[:, k, ts_],
                                     start=(k == 0), stop=(k == HJ - 1))
                ps3 = psum.tile([P, 512], f32, tag="ps3")
                for k in range(HJ):
                    nc.tensor.matmul(ps3[:], w3t[:, k, :], xT_sb[:, k, ts_],
                                     start=(k == 0), stop=(k == HJ - 1))
                sil = tmps.tile([P, 512], f32, tag="sil")
                nc.scalar.activation(sil[:], ps1[:], mybir.ActivationFunctionType.Silu)
                nc.vector.tensor_tensor(aT[:, f, ts_], sil[:], ps3[:],
                                        mybir.AluOpType.mult)

        # ---- Phase 3: y = a@w2.T, scaled by comb, to partial DRAM ----
        partial_dram = dram.tile([T, H], f32)
        for j in range(TT):
            for hh in range(2):
                hs = slice(hh * 512, (hh + 1) * 512)
                psy = psum.tile([P, 512], f32, tag="psy")
                for g in range(FG):
                    nc.tensor.matmul(psy[:], aT[:, g, j * P:(j + 1) * P],
                                     w2_sb[:, g, hs],
                                     start=(g == 0), stop=(g == FG - 1))
                yout = tmps.tile([P, 512], f32, tag="yout")
                nc.vector.tensor_scalar_mul(yout[:], psy[:], comb[:, j:j + 1])
                nc.sync.dma_start(partial_dram[j * P:(j + 1) * P, hs], yout[:])

        # ---- Phase 4: ReduceScatter(add) over the 8 cores ----
        if collective:
            rs_out = dram.tile([TSHARD, H], f32)
            nc.gpsimd.collective_compute(
                "ReduceScatter",
                mybir.AluOpType.add,
                replica_groups=[list(range(NCORES))],
                ins=[partial_dram[:].opt()],
                outs=[rs_out[:].opt()],
            )
            nc.sync.dma_start(final_shard[:], rs_out[:])
        else:
            nc.sync.dma_start(final_shard[:], partial_dram[:TSHARD, :])

    nc.compile()
    return nc


def _prep_inputs(hidden_states, gate_w, w1, w2, w3):
    """Host-side shard/layout prep (pure layout + dtype changes)."""
    x = np.asarray(hidden_states, np.float32).reshape(T, H)
    # xT[p, j, t] = x[t, j*128+p]
    xT = np.ascontiguousarray(x.reshape(T, HJ, P).transpose(2, 1, 0))
    xT_bf = np.ascontiguousarray(xT.astype(_BF16))
    # gateT[p, j, e] = gate_w[e, j*128+p]
    gateT = np.ascontiguousarray(
        np.asarray(gate_w, np.float32).reshape(E, HJ, P).transpose(2, 1, 0))

    w1 = np.asarray(w1, np.float32)
    w3 = np.asarray(w3, np.float32)
    w2 = np.asarray(w2, np.float32)
    in_maps = []
    for c in range(NCORES):
        # w1c[p, ft, j, fi] = w1[c, ft*128+fi, j*128+p]
        w1c = np.ascontiguousarray(
            w1[c].reshape(FG, P, HJ, P).transpose(3, 0, 2, 1).astype(_BF16))
        w3c = np.ascontiguousarray(
            w3[c].reshape(FG, P, HJ, P).transpose(3, 0, 2, 1).astype(_BF16))
        # w2c[p, g, h] = w2[c, h, g*128+p]
        w2c = np.ascontiguousarray(
            w2[c].reshape(H, FG, P).transpose(2, 1, 0).astype(_BF16))
        onehot = np.zeros((P, E), np.float32)
        onehot[:, c] = 1.0
        in_maps.append({
            "xT_bf": xT_bf,
            "xT_f32": xT,
            "gateT": gateT,
            "onehot": onehot,
            "w1c": w1c,
            "w3c": w3c,
            "w2c": w2c,
        })
    return in_maps


def kernel(hidden_states, gate_w, w1, w2, w3, trace=False):
    from concourse.bass_utils import run_bass_kernel_spmd

    if "nc" not in _compiled:
        _compiled["nc"] = _build_nc()
    nc = _compiled["nc"]

    in_maps = _prep_inputs(hidden_states, gate_w, w1, w2, w3)
    res = run_bass_kernel_spmd(nc, in_maps, core_ids=list(range(NCORES)),
                               trace=trace)
    _compiled["last_result"] = res

    shards = [res.results[c]["final_shard"] for c in range(NCORES)]
    final = np.concatenate(shards, axis=0).reshape(B, S, H).astype(np.float32)
    lg = res.results[0]["logits_out"]          # [p, j, e], t = j*128+p
    router_logits = np.ascontiguousarray(
        lg.transpose(1, 0, 2).reshape(T, E)).astype(np.float32)
    return final, router_logits
